# revision 25
# baseline (speedup 1.0000x reference)
"""CrossBlock kernel for trn2: 8-core data-parallel (batch x token-half).

Self-contained: builds a single SPMD Bass/Tile program, shards the full
inputs on the host, runs on 8 NeuronCores, reassembles full outputs.
"""

import os
import sys
from contextlib import ExitStack

import numpy as np

for _p in ("/opt/trn_rl_repo", os.path.expanduser("~/.axon_site/_ro/trn_rl_repo")):
    if os.path.isdir(_p) and _p not in sys.path:
        sys.path.insert(0, _p)

import concourse.bass as bass
import concourse.mybir as mybir
import concourse.tile as tile
from concourse.bass import ts

FP = mybir.dt.float32
AF = mybir.ActivationFunctionType
ALU = mybir.AluOpType

B, N_FULL, E, H, D = 4, 2048, 256, 4, 64
E2 = 2 * E
MULT = float(D ** -0.25)
EPS = 1e-5


def build_program(NT=N_FULL, gelu_func=None, cw=None, split_waits=True):
    """One core's program. NT = tokens per batch (full); the core owns the
    first NT//2 tokens of both sides ("own"), in its local order."""
    if gelu_func is None:
        gelu_func = AF.Gelu
    NHF = NT // 2          # tokens owned by this core per side
    RT = NHF // 128        # 128-row tiles over own tokens
    CW = cw if cw else min(1024, NT)       # col-block width
    CB = NT // CW                          # col blocks over full tokens
    MC = min(512, CW)      # matmul moving-operand chunk
    NMC = NHF // MC if NHF >= MC else 1    # chunks covering own half
    MCH = min(MC, NHF)

    nc = bass.Bass()

    # ---- I/O ----
    xin = {}
    for s in (0, 1):
        for part in ("own", "oth"):
            xin[(s, part)] = nc.declare_dram_parameter(
                f"x{s}_{part}", [NHF, E], FP, isOutput=False)
    wqkT = nc.declare_dram_parameter("wqkT", [E, E], FP, isOutput=False)
    wvT = nc.declare_dram_parameter("wvT", [E, E], FP, isOutput=False)
    woT = nc.declare_dram_parameter("woT", [E, E], FP, isOutput=False)
    w1T = nc.declare_dram_parameter("w1T", [E2, E2], FP, isOutput=False)
    w2T = nc.declare_dram_parameter("w2T", [E2, E], FP, isOutput=False)
    bqkm = nc.declare_dram_parameter("bqkm", [E, 1], FP, isOutput=False)
    bv_row = nc.declare_dram_parameter("bv_row", [1, E], FP, isOutput=False)
    bo_col = nc.declare_dram_parameter("bo_col", [E, 1], FP, isOutput=False)
    b1_row = nc.declare_dram_parameter("b1_row", [1, E2], FP, isOutput=False)
    b2_row = nc.declare_dram_parameter("b2_row", [1, E], FP, isOutput=False)
    g_row = nc.declare_dram_parameter("g_row", [1, E2], FP, isOutput=False)
    bl_row = nc.declare_dram_parameter("bl_row", [1, E2], FP, isOutput=False)
    ones_row = nc.declare_dram_parameter("ones_row", [1, 128], FP, isOutput=False)
    eps_col = nc.declare_dram_parameter("eps_col", [128, 1], FP, isOutput=False)
    ident = nc.declare_dram_parameter("ident", [128, 128], FP, isOutput=False)

    a01_o = nc.declare_dram_parameter("attn01_o", [H, NHF, NT], FP, isOutput=True)
    a10_o = nc.declare_dram_parameter("attn10_o", [H, NHF, NT], FP, isOutput=True)
    out0_o = nc.declare_dram_parameter("out0_o", [NHF, E], FP, isOutput=True)
    out1_o = nc.declare_dram_parameter("out1_o", [NHF, E], FP, isOutput=True)
    a_out = {0: a01_o, 1: a10_o}
    y_out = {0: out0_o, 1: out1_o}

    with tile.TileContext(nc) as tc, ExitStack() as ctx:
        P = lambda st, name, bufs, **kw: st.enter_context(
            tc.tile_pool(name=name, bufs=bufs, **kw))
        # ---- global pools (whole-kernel lifetime) ----
        const = P(ctx, "const", 1)
        xload = P(ctx, "xload", 3)
        xto_pool = P(ctx, "xto", 1)
        scol = P(ctx, "scol", 1)
        srow = P(ctx, "srow", 4)
        stat = P(ctx, "stat", 4)
        mtu = P(ctx, "mtu", 4)
        mts = P(ctx, "mts", 1)
        o_pool = P(ctx, "osb", 3)
        psum = P(ctx, "psum", 1, space="PSUM")

        dma = nc.sync.dma_start
        CWp = CW

        def ps1k(shape, name):
            return psum.tile(shape, FP, tag="ps1k", bufs=2, name=name,
                             padded_shape=[128, CWp])

        def ps512(shape, name):
            return psum.tile(shape, FP, tag="ps512", bufs=2, name=name,
                             padded_shape=[128, 512])

        def load_const(pool, ap, shape):
            t = pool.tile(shape, FP, tag=ap.name, name="c_" + ap.name)
            dma(t[:], ap[:])
            return t

        id_sb = load_const(const, ident, [128, 128])
        ones_sb = load_const(const, ones_row, [1, 128])
        eps_sb = load_const(const, eps_col, [128, 1])
        b1_sb = load_const(const, b1_row, [1, E2])
        b2_sb = load_const(const, b2_row, [1, E])

        # ---- phase 1 pools: attention-persistent (qt, v) then proj-scoped ----
        attn1_st = ExitStack()
        qt_pool = P(attn1_st, "qt", 1)
        v_pool = P(attn1_st, "v", 2 * (NT // 128))
        proj_st = ExitStack()
        pconst = P(proj_st, "pconst", 1)
        xtx_pool = P(proj_st, "xtx", 1)

        wqkT_sb = [pconst.tile([128, E], FP, tag=f"wqkT{k}", name=f"wqkT{k}")
                   for k in range(2)]
        wvT_sb = [pconst.tile([128, E], FP, tag=f"wvT{k}", name=f"wvT{k}")
                  for k in range(2)]
        for k in range(2):
            dma(wqkT_sb[k][:], wqkT[ts(k, 128), :])
            dma(wvT_sb[k][:], wvT[ts(k, 128), :])
        bqkm_sb = [pconst.tile([128, 1], FP, tag=f"bqkm{k}", name=f"bqkm{k}")
                   for k in range(2)]
        for k in range(2):
            dma(bqkm_sb[k][:], bqkm[ts(k, 128), :])
        bv_sb = load_const(pconst, bv_row, [1, E])

        # ---- load x, build X^T (feature-major, local token order) ----
        xto, xtx = {}, {}
        for s in (0, 1):
            for k in range(2):
                xto[(s, k)] = xto_pool.tile([128, NHF], FP, tag=f"xto{s}{k}",
                                            name=f"xto{s}{k}")
                xtx[(s, k)] = xtx_pool.tile([128, NHF], FP, tag=f"xtx{s}{k}",
                                            name=f"xtx{s}{k}")
            for part, dst in (("own", xto), ("oth", xtx)):
                for t in range(RT):
                    xtile = xload.tile([128, E], FP, tag="xl")
                    dma(xtile[:], xin[(s, part)][ts(t, 128), :])
                    for k in range(2):
                        tp = ps512([128, 128], "tp")
                        nc.tensor.transpose(tp[:], xtile[:, ts(k, 128)], id_sb[:])
                        nc.scalar.copy(dst[(s, k)][:, ts(t, 128)], tp[:])

        def xt_at(s, k, c0, w):
            if c0 < NHF:
                assert c0 + w <= NHF
                return xto[(s, k)][:, bass.ds(c0, w)]
            return xtx[(s, k)][:, bass.ds(c0 - NHF, w)]

        # ---- projections ----
        qt = {}
        for s in (0, 1):
            for p in range(2):
                qt[(s, p)] = qt_pool.tile([128, NT], FP, tag=f"qt{s}{p}",
                                          name=f"qt{s}{p}")
                MCQ = min(MC, NHF)
                for cc in range(NT // MCQ):
                    ps = ps512([128, MCQ], "qtps")
                    for k in range(2):
                        nc.tensor.matmul(
                            ps[:], wqkT_sb[k][:, ts(p, 128)],
                            xt_at(s, k, cc * MCQ, MCQ),
                            start=(k == 0), stop=(k == 1))
                    nc.scalar.activation(
                        qt[(s, p)][:, ts(cc, MCQ)], ps[:], AF.Identity,
                        bias=bqkm_sb[p][:], scale=MULT)
        vt = {}
        for s in (0, 1):
            for t in range(NT // 128):
                ps = ps512([128, E], "vps")
                nc.tensor.matmul(ps[:], ones_sb[:, :], bv_sb[:, :],
                                 start=True, stop=False)
                for k in range(2):
                    nc.tensor.matmul(ps[:], xt_at(s, k, t * 128, 128),
                                     wvT_sb[k][:], start=False, stop=(k == 1))
                v = v_pool.tile([128, E], FP, tag="vsb", name=f"v{s}_{t}")
                nc.vector.tensor_copy(v[:], ps[:])
                vt[(s, t)] = v
        proj_st.close()   # release X^T "other" + projection weights

        # ---- attention-transient pools (created after proj area freed) ----
        attn2_st = ExitStack()
        ebig = P(attn2_st, "ebig", 2)
        esmall = P(attn2_st, "esmall", 2)
        aout = P(attn2_st, "aout", 2)

        # orientation o=0: rows side0 (attn01), cols side1, msg m1 (for x1)
        # orientation o=1: rows side1 (attn10), cols side0, msg m0 (for x0)
        mt = {}
        for pair in range(2):
            m_unscaled = {}
            rs_rows = {}
            for o in (0, 1):
                rows_s, cols_s = (0, 1) if o == 0 else (1, 0)
                m_acc = psum.tile([128, NHF], FP, tag="macc", bufs=1,
                                  name="m_acc")
                s_cols = {h: scol.tile([128, RT], FP, tag=f"sc{o}{pair}{h}",
                                       name=f"sc{o}{pair}{h}", bufs=1)
                          for h in range(2)}
                # main pass: rows = own tokens of rows_s, cols = all
                for t in range(RT):
                    for h in range(2):
                        head = 2 * pair + h
                        hsl = bass.ds(64 * h, 64)
                        etile = ebig.tile([128, NT], FP, tag="ebig",
                                          name="etile")
                        spc = stat.tile([128, CB], FP, tag="spc", name="spc")
                        for cb in range(CB):
                            sps = ps1k([128, CW], "sps")
                            for c in range(CW // MC):
                                nc.tensor.matmul(
                                    sps[:, ts(c, MC)],
                                    qt[(rows_s, pair)][hsl, ts(t, 128)],
                                    qt[(cols_s, pair)][hsl,
                                        bass.ds(cb * CW + c * MC, MC)],
                                    start=True, stop=True)
                            nc.scalar.activation(
                                etile[:, ts(cb, CW)], sps[:], AF.Exp,
                                accum_out=spc[:, cb:cb + 1])
                        if CB > 1:
                            nc.vector.tensor_add(
                                s_cols[h][:, t:t + 1], spc[:, 0:1], spc[:, 1:2])
                        else:
                            nc.vector.tensor_copy(
                                s_cols[h][:, t:t + 1], spc[:, 0:1])
                        rc = stat.tile([128, 1], FP, tag="rc", name="rc")
                        nc.vector.reciprocal(rc[:], s_cols[h][:, t:t + 1])
                        at = aout.tile([128, NT], FP, tag="aout", name="at")
                        nc.vector.tensor_scalar_mul(at[:], etile[:], rc[:])
                        dma(a_out[o][head, ts(t, 128), :], at[:])
                        for c in range(NMC):
                            nc.tensor.matmul(
                                m_acc[bass.ds(64 * h, 64), ts(c, MCH)],
                                vt[(rows_s, t)][:, bass.ds(64 * head, 64)],
                                etile[:, ts(c, MCH)],
                                start=(t == 0), stop=False,
                                tile_position=(0, 64 * h),
                                skip_group_check=True)
                # completion pass: rows = other tokens of rows_s, cols = own
                for t in range(RT):
                    for h in range(2):
                        head = 2 * pair + h
                        hsl = bass.ds(64 * h, 64)
                        et2 = esmall.tile([128, NHF], FP, tag="esm",
                                          name="et2")
                        for cb in range((NHF + CW - 1) // CW):
                            w = min(CW, NHF - cb * CW)
                            sps = ps1k([128, CW], "sps")
                            for c in range(w // MC if w >= MC else 1):
                                mc2 = min(MC, w)
                                nc.tensor.matmul(
                                    sps[:, ts(c, mc2)],
                                    qt[(rows_s, pair)][hsl,
                                        bass.ds(NHF + t * 128, 128)],
                                    qt[(cols_s, pair)][hsl,
                                        bass.ds(cb * CW + c * mc2, mc2)],
                                    start=True, stop=True)
                            nc.scalar.activation(
                                et2[:, bass.ds(cb * CW, w)],
                                sps[:, bass.ds(0, w)], AF.Exp)
                        last = (t == RT - 1)
                        for c in range(NMC):
                            nc.tensor.matmul(
                                m_acc[bass.ds(64 * h, 64), ts(c, MCH)],
                                vt[(rows_s, RT + t)][:, bass.ds(64 * head, 64)],
                                et2[:, ts(c, MCH)],
                                start=False, stop=last,
                                tile_position=(0, 64 * h),
                                skip_group_check=True)
                # evacuate unnormalized message; recip row-sums as [1, NHF]
                mu = mtu.tile([128, NHF], FP, tag="mtu", name="mu")
                nc.vector.tensor_copy(mu[:], m_acc[:])
                m_unscaled[o] = mu
                for h in range(2):
                    head = 2 * pair + h
                    rcc = scol.tile([128, RT], FP, tag=f"rcc{o}{pair}{h}",
                                    name=f"rcc{o}{pair}{h}", bufs=1)
                    nc.vector.reciprocal(rcc[:], s_cols[h][:])
                    tp = ps512([RT, 128], "sctp")
                    nc.tensor.transpose(tp[:], rcc[:], id_sb[:])
                    t8 = stat.tile([RT, 128], FP, tag="t8", name="t8")
                    nc.vector.tensor_copy(t8[:], tp[:])
                    rsr = srow.tile([1, NHF], FP, tag="rsr", name="rsr")
                    dma(rsr[0:1, :], t8[:, :])
                    rs_rows[(o, head)] = rsr
            # scale this pair's messages by the opposite orientation's sums
            for o in (0, 1):
                bc = ps1k([128, NHF], "bc")
                for h in range(2):
                    head = 2 * pair + h
                    for t in range(RT):
                        nc.tensor.matmul(
                            bc[bass.ds(64 * h, 64), ts(t, 128)],
                            ones_sb[:, 0:64],
                            rs_rows[(1 - o, head)][0:1, ts(t, 128)],
                            start=True, stop=True,
                            tile_position=(0, 64 * h))
                msc = mts.tile([128, NHF], FP, tag=f"mts{o}{pair}",
                               name=f"mts{o}{pair}", bufs=1)
                nc.vector.tensor_mul(msc[:], m_unscaled[o][:], bc[:])
                mt[(o, pair)] = msc
        attn2_st.close()
        attn1_st.close()

        # ---- FFN phase ----
        ffn_st = ExitStack()
        fconst = P(ffn_st, "fconst", 1)
        mo_pool = P(ffn_st, "mo", 1)
        h_pool = P(ffn_st, "hsb", 1)
        hg_pool = P(ffn_st, "hg", 2)
        hgt_pool = P(ffn_st, "hgt", 2)
        woT_sb = [fconst.tile([128, E], FP, tag=f"woT{k}", name=f"woT{k}")
                  for k in range(2)]
        bo_sb = [fconst.tile([128, 1], FP, tag=f"bo{k}", name=f"bo{k}")
                 for k in range(2)]
        for k in range(2):
            dma(woT_sb[k][:], woT[ts(k, 128), :])
            dma(bo_sb[k][:], bo_col[ts(k, 128), :])
        w1T_sb = [fconst.tile([128, E2], FP, tag=f"w1T{k}", name=f"w1T{k}")
                  for k in range(4)]
        w2T_sb = [fconst.tile([128, E], FP, tag=f"w2T{k}", name=f"w2T{k}")
                  for k in range(4)]
        for k in range(4):
            dma(w1T_sb[k][:], w1T[ts(k, 128), :])
            dma(w2T_sb[k][:], w2T[ts(k, 128), :])
        g_sb1 = load_const(fconst, g_row, [1, E2])
        bl_sb1 = load_const(fconst, bl_row, [1, E2])
        gb_ps = ps512([128, E2], "gb_ps")
        for cc in range(E2 // 512):
            nc.tensor.matmul(gb_ps[:, ts(cc, 512)], ones_sb[:, :],
                             g_sb1[:, ts(cc, 512)], start=True, stop=True)
        g_bc = fconst.tile([128, E2], FP, tag="g_bc")
        nc.vector.tensor_copy(g_bc[:], gb_ps[:])
        bb_ps = ps512([128, E2], "bb_ps")
        for cc in range(E2 // 512):
            nc.tensor.matmul(bb_ps[:, ts(cc, 512)], ones_sb[:, :],
                             bl_sb1[:, ts(cc, 512)], start=True, stop=True)
        b_bc = fconst.tile([128, E2], FP, tag="b_bc")
        nc.vector.tensor_copy(b_bc[:], bb_ps[:])

        # side 0 uses m from orientation o=1 (m0); side 1 uses o=0 (m1)
        moT = {}
        for s in (0, 1):
            o = 1 - s
            for p in range(2):
                mo_sb = mo_pool.tile([128, NHF], FP, tag=f"mo{s}{p}",
                                     name=f"mo{s}{p}", bufs=1)
                for cc in range(NMC):
                    ps = ps512([128, MCH], "mops")
                    for k in range(2):
                        nc.tensor.matmul(
                            ps[:], woT_sb[k][:, ts(p, 128)],
                            mt[(o, k)][:, ts(cc, MCH)],
                            start=(k == 0), stop=(k == 1))
                    nc.scalar.activation(
                        mo_sb[:, ts(cc, MCH)], ps[:], AF.Identity,
                        bias=bo_sb[p][:], scale=1.0)
                moT[(s, p)] = mo_sb

        # sweep A: H = W1 @ [x;m] + b1 (token-major), center, rstd
        h_sb = {}
        rstd = {}
        for s in (0, 1):
            ctk = [xto[(s, 0)], xto[(s, 1)], moT[(s, 0)], moT[(s, 1)]]
            for j in range(RT):
                hp = ps512([128, E2], "hp")
                nc.tensor.matmul(hp[:, 0:E2], ones_sb[:, :], b1_sb[:, 0:E2],
                                 start=True, stop=False)
                for k in range(4):
                    nc.tensor.matmul(hp[:, 0:E2], ctk[k][:, ts(j, 128)],
                                     w1T_sb[k][:], start=False, stop=(k == 3))
                st6 = stat.tile([128, 6], FP, tag="st6", name="st6")
                nc.vector.bn_stats(st6[:], hp[:])
                mv = stat.tile([128, 2], FP, tag="mv", name="mv")
                nc.vector.bn_aggr(mv[:], st6[:])
                hs = h_pool.tile([128, E2], FP, tag=f"h{s}{j}",
                                 name=f"h{s}{j}", bufs=1)
                nc.vector.tensor_scalar(
                    hs[:], hp[:], mv[:, 0:1], None, op0=ALU.subtract)
                h_sb[(s, j)] = hs
                sq = stat.tile([128, 1], FP, tag="sq", name="sq")
                nc.scalar.activation(sq[:], mv[:, 1:2], AF.Sqrt, bias=eps_sb[:])
                rc = stat.tile([128, 1], FP, tag=f"rstd{s}{j}",
                               name=f"rstd{s}{j}", bufs=1)
                nc.vector.reciprocal(rc[:], sq[:])
                rstd[(s, j)] = rc

        # sweep B: scale*g+b, gelu, transpose, W2, +x, store
        for s in (0, 1):
            for j in range(RT):
                hn = hg_pool.tile([128, E2], FP, tag="hn", name="hn")
                nc.vector.tensor_scalar_mul(hn[:], h_sb[(s, j)][:],
                                            rstd[(s, j)][:])
                nc.vector.tensor_mul(hn[:], hn[:], g_bc[:])
                nc.vector.tensor_add(hn[:], hn[:], b_bc[:])
                hg = hg_pool.tile([128, E2], FP, tag="hg", name="hg")
                nc.scalar.activation(hg[:], hn[:], gelu_func)
                hgt = [hgt_pool.tile([128, 128], FP, tag=f"hgt{k}",
                                     name=f"hgt{k}") for k in range(4)]
                for k in range(4):
                    tp = ps512([128, 128], "tp2")
                    nc.tensor.transpose(tp[:], hg[:, ts(k, 128)], id_sb[:])
                    nc.scalar.copy(hgt[k][:], tp[:])
                op = ps512([128, E], "op")
                nc.tensor.matmul(op[:], ones_sb[:, :], b2_sb[:, :],
                                 start=True, stop=False)
                for k in range(4):
                    nc.tensor.matmul(op[:], hgt[k][:], w2T_sb[k][:],
                                     start=False, stop=(k == 3))
                xres = xload.tile([128, E], FP, tag="xl", name="xres")
                dma(xres[:], xin[(s, "own")][ts(j, 128), :])
                yo = o_pool.tile([128, E], FP, tag="yo", name="yo")
                nc.vector.tensor_add(yo[:], op[:], xres[:])
                dma(y_out[s][ts(j, 128), :], yo[:])
        ffn_st.close()

    if split_waits:
        _split_matmul_waits(nc)
    return nc


_SPLIT_TYPES = ("InstMatmult", "InstDMACopy", "InstDMATranspose")


def _split_matmul_waits(nc, max_waits=1):
    """walrus (this build) rejects Matmult instructions carrying more than one
    sync wait (S3_LW struct). Strip waits off such matmuls onto single-wait
    EventSemaphore instructions inserted just before them on the same engine."""
    esn = [0]
    for blk in nc.m.functions[0].blocks:
        insts = blk.instructions   # live list
        i = 0
        while i < len(insts):
            inst = insts[i]
            si = inst.sync_info
            if (type(inst).__name__ != "InstEventSemaphore" and si is not None
                    and len(si.on_wait) > max_waits):
                waits = list(si.on_wait)
                inst.sync_info = mybir.SyncInfo(on_wait=[], on_update=list(si.on_update))
                for w in waits:
                    es = mybir.InstEventSemaphore(name=f"I-esw{esn[0]}", ins=[], outs=[])
                    esn[0] += 1
                    es.engine = inst.engine
                    es.sync_info = mybir.SyncInfo(on_wait=[w], on_update=[])
                    insts.insert(i, es)
                    i += 1
            i += 1
    return nc


_CACHE = {}


def _get_nc(NT):
    if NT not in _CACHE:
        _CACHE[NT] = build_program(NT)
    return _CACHE[NT]


def make_in_maps(inputs, NT=N_FULL, n_cores=8):
    """Host-side sharding: per-core input dicts."""
    f32 = lambda a: np.ascontiguousarray(np.asarray(a, dtype=np.float32))
    x0, x1 = f32(inputs["x0"]), f32(inputs["x1"])
    Wqk, bqk = f32(inputs["Wqk"]), f32(inputs["bqk"])
    Wv, bv = f32(inputs["Wv"]), f32(inputs["bv"])
    Wo, bo = f32(inputs["Wo"]), f32(inputs["bo"])
    W1, b1 = f32(inputs["W1"]), f32(inputs["b1"])
    g_ln, b_ln = f32(inputs["g_ln"]), f32(inputs["b_ln"])
    W2, b2 = f32(inputs["W2"]), f32(inputs["b2"])
    NHF = NT // 2
    shared = dict(
        wqkT=f32(Wqk.T), wvT=f32(Wv.T), woT=f32(Wo.T),
        w1T=f32(W1.T), w2T=f32(W2.T),
        bqkm=f32((bqk * MULT).reshape(E, 1)),
        bv_row=f32(bv.reshape(1, E)),
        bo_col=f32(bo.reshape(E, 1)),
        b1_row=f32(b1.reshape(1, E2)),
        b2_row=f32(b2.reshape(1, E)),
        g_row=f32(g_ln.reshape(1, E2)),
        bl_row=f32(b_ln.reshape(1, E2)),
        ones_row=np.ones((1, 128), np.float32),
        eps_col=np.full((128, 1), EPS, np.float32),
        ident=np.eye(128, dtype=np.float32),
    )
    in_maps = []
    for c in range(n_cores):
        b, hf = divmod(c, 2)
        own = slice(hf * NHF, (hf + 1) * NHF)
        oth = slice((1 - hf) * NHF, (2 - hf) * NHF)
        m = dict(shared)
        m["x0_own"] = f32(x0[b, own])
        m["x0_oth"] = f32(x0[b, oth])
        m["x1_own"] = f32(x1[b, own])
        m["x1_oth"] = f32(x1[b, oth])
        in_maps.append(m)
    return in_maps


def assemble(results, NT=N_FULL, n_cores=8):
    """Reassemble per-core outputs into full arrays."""
    NHF = NT // 2
    out0 = np.empty((B, NT, E), np.float32)
    out1 = np.empty((B, NT, E), np.float32)
    a01 = np.empty((B, H, NT, NT), np.float32)
    a10 = np.empty((B, H, NT, NT), np.float32)
    for c in range(n_cores):
        b, hf = divmod(c, 2)
        own = slice(hf * NHF, (hf + 1) * NHF)
        oth = slice((1 - hf) * NHF, (2 - hf) * NHF)
        r = results[c]
        out0[b, own] = r["out0_o"]
        out1[b, own] = r["out1_o"]
        a01[b, :, own, own] = r["attn01_o"][:, :, :NHF]
        a01[b, :, own, oth] = r["attn01_o"][:, :, NHF:]
        a10[b, :, own, own] = r["attn10_o"][:, :, :NHF]
        a10[b, :, own, oth] = r["attn10_o"][:, :, NHF:]
    return out0, out1, a01, a10


LAST_RUN = None


def kernel(**inputs):
    global LAST_RUN
    os.environ.setdefault("MYCRO_LOCAL_CACHE", "1")
    from concourse.bass_utils import run_bass_kernel_spmd
    nc = _get_nc(N_FULL)
    in_maps = make_in_maps(inputs, N_FULL, 8)
    res = run_bass_kernel_spmd(nc, in_maps, core_ids=list(range(8)))
    LAST_RUN = res
    return assemble(res.results, N_FULL, 8)


# revision 26
# speedup vs baseline: 21212.2026x; 21212.2026x over previous
"""CrossBlock kernel for trn2: 8-core data-parallel (batch x token-half).

Self-contained: builds a single SPMD Bass/Tile program, shards the full
inputs on the host, runs on 8 NeuronCores, reassembles full outputs.
"""

import os
import sys
from contextlib import ExitStack

import numpy as np

for _p in ("/opt/trn_rl_repo", os.path.expanduser("~/.axon_site/_ro/trn_rl_repo")):
    if os.path.isdir(_p) and _p not in sys.path:
        sys.path.insert(0, _p)

import concourse.bass as bass
import concourse.mybir as mybir
import concourse.tile as tile
from concourse.bass import ts

FP = mybir.dt.float32
AF = mybir.ActivationFunctionType
ALU = mybir.AluOpType

B, N_FULL, E, H, D = 4, 2048, 256, 4, 64
E2 = 2 * E
MULT = float(D ** -0.25)
EPS = 1e-5


def build_program(NT=N_FULL, gelu_func=None, cw=None, split_waits=True):
    """One core's program. NT = tokens per batch (full); the core owns the
    first NT//2 tokens of both sides ("own"), in its local order."""
    if gelu_func is None:
        gelu_func = AF.Gelu
    NHF = NT // 2          # tokens owned by this core per side
    RT = NHF // 128        # 128-row tiles over own tokens
    CW = cw if cw else min(1024, NT)       # col-block width
    CB = NT // CW                          # col blocks over full tokens
    MC = min(512, CW)      # matmul moving-operand chunk
    NMC = NHF // MC if NHF >= MC else 1    # chunks covering own half
    MCH = min(MC, NHF)

    nc = bass.Bass()

    # ---- I/O ----
    xin = {}
    for s in (0, 1):
        for part in ("own", "oth"):
            xin[(s, part)] = nc.declare_dram_parameter(
                f"x{s}_{part}", [NHF, E], FP, isOutput=False)
    wqkT = nc.declare_dram_parameter("wqkT", [E, E], FP, isOutput=False)
    wvT = nc.declare_dram_parameter("wvT", [E, E], FP, isOutput=False)
    woT = nc.declare_dram_parameter("woT", [E, E], FP, isOutput=False)
    w1T = nc.declare_dram_parameter("w1T", [E2, E2], FP, isOutput=False)
    w2T = nc.declare_dram_parameter("w2T", [E2, E], FP, isOutput=False)
    bqkm = nc.declare_dram_parameter("bqkm", [E, 1], FP, isOutput=False)
    bv_row = nc.declare_dram_parameter("bv_row", [1, E], FP, isOutput=False)
    bo_col = nc.declare_dram_parameter("bo_col", [E, 1], FP, isOutput=False)
    b1_row = nc.declare_dram_parameter("b1_row", [1, E2], FP, isOutput=False)
    b2_row = nc.declare_dram_parameter("b2_row", [1, E], FP, isOutput=False)
    g_row = nc.declare_dram_parameter("g_row", [1, E2], FP, isOutput=False)
    bl_row = nc.declare_dram_parameter("bl_row", [1, E2], FP, isOutput=False)
    ones_row = nc.declare_dram_parameter("ones_row", [1, 128], FP, isOutput=False)
    eps_col = nc.declare_dram_parameter("eps_col", [128, 1], FP, isOutput=False)
    ident = nc.declare_dram_parameter("ident", [128, 128], FP, isOutput=False)

    a01_o = nc.declare_dram_parameter("attn01_o", [H, NHF, NT], FP, isOutput=True)
    a10_o = nc.declare_dram_parameter("attn10_o", [H, NHF, NT], FP, isOutput=True)
    out0_o = nc.declare_dram_parameter("out0_o", [NHF, E], FP, isOutput=True)
    out1_o = nc.declare_dram_parameter("out1_o", [NHF, E], FP, isOutput=True)
    a_out = {0: a01_o, 1: a10_o}
    y_out = {0: out0_o, 1: out1_o}

    with tile.TileContext(nc) as tc, ExitStack() as ctx:
        P = lambda st, name, bufs, **kw: st.enter_context(
            tc.tile_pool(name=name, bufs=bufs, **kw))
        # ---- global pools (whole-kernel lifetime) ----
        const = P(ctx, "const", 1)
        xload = P(ctx, "xload", 3)
        xto_pool = P(ctx, "xto", 1)
        scol = P(ctx, "scol", 1)
        srow = P(ctx, "srow", 4)
        stat = P(ctx, "stat", 4)
        mtu = P(ctx, "mtu", 4)
        mts = P(ctx, "mts", 1)
        o_pool = P(ctx, "osb", 3)
        psum = P(ctx, "psum", 1, space="PSUM")

        dma = nc.sync.dma_start
        CWp = CW

        def ps1k(shape, name):
            return psum.tile(shape, FP, tag="ps1k", bufs=2, name=name,
                             padded_shape=[128, CWp])

        def ps512(shape, name):
            return psum.tile(shape, FP, tag="ps512", bufs=2, name=name,
                             padded_shape=[128, 512])

        def load_const(pool, ap, shape):
            t = pool.tile(shape, FP, tag=ap.name, name="c_" + ap.name)
            dma(t[:], ap[:])
            return t

        id_sb = load_const(const, ident, [128, 128])
        ones_sb = load_const(const, ones_row, [1, 128])
        eps_sb = load_const(const, eps_col, [128, 1])
        b1_sb = load_const(const, b1_row, [1, E2])
        b2_sb = load_const(const, b2_row, [1, E])

        # ---- phase 1 pools: attention-persistent (qt, v) then proj-scoped ----
        attn1_st = ExitStack()
        qt_pool = P(attn1_st, "qt", 1)
        v_pool = P(attn1_st, "v", 2 * (NT // 128))
        proj_st = ExitStack()
        pconst = P(proj_st, "pconst", 1)
        xtx_pool = P(proj_st, "xtx", 1)

        wqkT_sb = [pconst.tile([128, E], FP, tag=f"wqkT{k}", name=f"wqkT{k}")
                   for k in range(2)]
        wvT_sb = [pconst.tile([128, E], FP, tag=f"wvT{k}", name=f"wvT{k}")
                  for k in range(2)]
        for k in range(2):
            dma(wqkT_sb[k][:], wqkT[ts(k, 128), :])
            dma(wvT_sb[k][:], wvT[ts(k, 128), :])
        bqkm_sb = [pconst.tile([128, 1], FP, tag=f"bqkm{k}", name=f"bqkm{k}")
                   for k in range(2)]
        for k in range(2):
            dma(bqkm_sb[k][:], bqkm[ts(k, 128), :])
        bv_sb = load_const(pconst, bv_row, [1, E])

        # ---- load x, build X^T (feature-major, local token order) ----
        xto, xtx = {}, {}
        for s in (0, 1):
            for k in range(2):
                xto[(s, k)] = xto_pool.tile([128, NHF], FP, tag=f"xto{s}{k}",
                                            name=f"xto{s}{k}")
                xtx[(s, k)] = xtx_pool.tile([128, NHF], FP, tag=f"xtx{s}{k}",
                                            name=f"xtx{s}{k}")
            for part, dst in (("own", xto), ("oth", xtx)):
                for t in range(RT):
                    xtile = xload.tile([128, E], FP, tag="xl")
                    dma(xtile[:], xin[(s, part)][ts(t, 128), :])
                    for k in range(2):
                        tp = ps512([128, 128], "tp")
                        nc.tensor.transpose(tp[:], xtile[:, ts(k, 128)], id_sb[:])
                        nc.scalar.copy(dst[(s, k)][:, ts(t, 128)], tp[:])

        def xt_at(s, k, c0, w):
            if c0 < NHF:
                assert c0 + w <= NHF
                return xto[(s, k)][:, bass.ds(c0, w)]
            return xtx[(s, k)][:, bass.ds(c0 - NHF, w)]

        # ---- projections ----
        qt = {}
        for s in (0, 1):
            for p in range(2):
                qt[(s, p)] = qt_pool.tile([128, NT], FP, tag=f"qt{s}{p}",
                                          name=f"qt{s}{p}")
                MCQ = min(MC, NHF)
                for cc in range(NT // MCQ):
                    ps = ps512([128, MCQ], "qtps")
                    for k in range(2):
                        nc.tensor.matmul(
                            ps[:], wqkT_sb[k][:, ts(p, 128)],
                            xt_at(s, k, cc * MCQ, MCQ),
                            start=(k == 0), stop=(k == 1))
                    nc.scalar.activation(
                        qt[(s, p)][:, ts(cc, MCQ)], ps[:], AF.Identity,
                        bias=bqkm_sb[p][:], scale=MULT)
        vt = {}
        for s in (0, 1):
            for t in range(NT // 128):
                ps = ps512([128, E], "vps")
                nc.tensor.matmul(ps[:], ones_sb[:, :], bv_sb[:, :],
                                 start=True, stop=False)
                for k in range(2):
                    nc.tensor.matmul(ps[:], xt_at(s, k, t * 128, 128),
                                     wvT_sb[k][:], start=False, stop=(k == 1))
                v = v_pool.tile([128, E], FP, tag="vsb", name=f"v{s}_{t}")
                nc.vector.tensor_copy(v[:], ps[:])
                vt[(s, t)] = v
        proj_st.close()   # release X^T "other" + projection weights

        # ---- attention-transient pools (created after proj area freed) ----
        attn2_st = ExitStack()
        ebig = P(attn2_st, "ebig", 2)
        esmall = P(attn2_st, "esmall", 2)
        aout = P(attn2_st, "aout", 2)

        # orientation o=0: rows side0 (attn01), cols side1, msg m1 (for x1)
        # orientation o=1: rows side1 (attn10), cols side0, msg m0 (for x0)
        mt = {}
        for pair in range(2):
            m_unscaled = {}
            rs_rows = {}
            for o in (0, 1):
                rows_s, cols_s = (0, 1) if o == 0 else (1, 0)
                m_acc = psum.tile([128, NHF], FP, tag="macc", bufs=1,
                                  name="m_acc")
                s_cols = {h: scol.tile([128, RT], FP, tag=f"sc{o}{pair}{h}",
                                       name=f"sc{o}{pair}{h}", bufs=1)
                          for h in range(2)}
                # main pass: rows = own tokens of rows_s, cols = all
                for t in range(RT):
                    for h in range(2):
                        head = 2 * pair + h
                        hsl = bass.ds(64 * h, 64)
                        etile = ebig.tile([128, NT], FP, tag="ebig",
                                          name="etile")
                        spc = stat.tile([128, CB], FP, tag="spc", name="spc")
                        for cb in range(CB):
                            sps = ps1k([128, CW], "sps")
                            for c in range(CW // MC):
                                nc.tensor.matmul(
                                    sps[:, ts(c, MC)],
                                    qt[(rows_s, pair)][hsl, ts(t, 128)],
                                    qt[(cols_s, pair)][hsl,
                                        bass.ds(cb * CW + c * MC, MC)],
                                    start=True, stop=True)
                            nc.scalar.activation(
                                etile[:, ts(cb, CW)], sps[:], AF.Exp,
                                accum_out=spc[:, cb:cb + 1])
                        if CB > 1:
                            nc.vector.tensor_add(
                                s_cols[h][:, t:t + 1], spc[:, 0:1], spc[:, 1:2])
                        else:
                            nc.vector.tensor_copy(
                                s_cols[h][:, t:t + 1], spc[:, 0:1])
                        rc = stat.tile([128, 1], FP, tag="rc", name="rc")
                        nc.vector.reciprocal(rc[:], s_cols[h][:, t:t + 1])
                        at = aout.tile([128, NT], FP, tag="aout", name="at")
                        nc.vector.tensor_scalar_mul(at[:], etile[:], rc[:])
                        dma(a_out[o][head, ts(t, 128), :], at[:])
                        for c in range(NMC):
                            nc.tensor.matmul(
                                m_acc[bass.ds(64 * h, 64), ts(c, MCH)],
                                vt[(rows_s, t)][:, bass.ds(64 * head, 64)],
                                etile[:, ts(c, MCH)],
                                start=(t == 0), stop=False,
                                tile_position=(0, 64 * h),
                                skip_group_check=True)
                # completion pass: rows = other tokens of rows_s, cols = own
                for t in range(RT):
                    for h in range(2):
                        head = 2 * pair + h
                        hsl = bass.ds(64 * h, 64)
                        et2 = esmall.tile([128, NHF], FP, tag="esm",
                                          name="et2")
                        for cb in range((NHF + CW - 1) // CW):
                            w = min(CW, NHF - cb * CW)
                            sps = ps1k([128, CW], "sps")
                            for c in range(w // MC if w >= MC else 1):
                                mc2 = min(MC, w)
                                nc.tensor.matmul(
                                    sps[:, ts(c, mc2)],
                                    qt[(rows_s, pair)][hsl,
                                        bass.ds(NHF + t * 128, 128)],
                                    qt[(cols_s, pair)][hsl,
                                        bass.ds(cb * CW + c * mc2, mc2)],
                                    start=True, stop=True)
                            nc.scalar.activation(
                                et2[:, bass.ds(cb * CW, w)],
                                sps[:, bass.ds(0, w)], AF.Exp)
                        last = (t == RT - 1)
                        for c in range(NMC):
                            nc.tensor.matmul(
                                m_acc[bass.ds(64 * h, 64), ts(c, MCH)],
                                vt[(rows_s, RT + t)][:, bass.ds(64 * head, 64)],
                                et2[:, ts(c, MCH)],
                                start=False, stop=last,
                                tile_position=(0, 64 * h),
                                skip_group_check=True)
                # evacuate unnormalized message; recip row-sums as [1, NHF]
                mu = mtu.tile([128, NHF], FP, tag="mtu", name="mu")
                nc.vector.tensor_copy(mu[:], m_acc[:])
                m_unscaled[o] = mu
                for h in range(2):
                    head = 2 * pair + h
                    rcc = scol.tile([128, RT], FP, tag=f"rcc{o}{pair}{h}",
                                    name=f"rcc{o}{pair}{h}", bufs=1)
                    nc.vector.reciprocal(rcc[:], s_cols[h][:])
                    tp = ps512([RT, 128], "sctp")
                    nc.tensor.transpose(tp[:], rcc[:], id_sb[:])
                    t8 = stat.tile([RT, 128], FP, tag="t8", name="t8")
                    nc.vector.tensor_copy(t8[:], tp[:])
                    rsr = srow.tile([1, NHF], FP, tag="rsr", name="rsr")
                    dma(rsr[0:1, :], t8[:, :])
                    rs_rows[(o, head)] = rsr
            # scale this pair's messages by the opposite orientation's sums
            for o in (0, 1):
                bc = ps1k([128, NHF], "bc")
                for h in range(2):
                    head = 2 * pair + h
                    for t in range(RT):
                        nc.tensor.matmul(
                            bc[bass.ds(64 * h, 64), ts(t, 128)],
                            ones_sb[:, 0:64],
                            rs_rows[(1 - o, head)][0:1, ts(t, 128)],
                            start=True, stop=True,
                            tile_position=(0, 64 * h))
                msc = mts.tile([128, NHF], FP, tag=f"mts{o}{pair}",
                               name=f"mts{o}{pair}", bufs=1)
                nc.vector.tensor_mul(msc[:], m_unscaled[o][:], bc[:])
                mt[(o, pair)] = msc
        attn2_st.close()
        attn1_st.close()

        # ---- FFN phase ----
        ffn_st = ExitStack()
        fconst = P(ffn_st, "fconst", 1)
        mo_pool = P(ffn_st, "mo", 1)
        h_pool = P(ffn_st, "hsb", 1)
        hg_pool = P(ffn_st, "hg", 2)
        hgt_pool = P(ffn_st, "hgt", 2)
        woT_sb = [fconst.tile([128, E], FP, tag=f"woT{k}", name=f"woT{k}")
                  for k in range(2)]
        bo_sb = [fconst.tile([128, 1], FP, tag=f"bo{k}", name=f"bo{k}")
                 for k in range(2)]
        for k in range(2):
            dma(woT_sb[k][:], woT[ts(k, 128), :])
            dma(bo_sb[k][:], bo_col[ts(k, 128), :])
        w1T_sb = [fconst.tile([128, E2], FP, tag=f"w1T{k}", name=f"w1T{k}")
                  for k in range(4)]
        w2T_sb = [fconst.tile([128, E], FP, tag=f"w2T{k}", name=f"w2T{k}")
                  for k in range(4)]
        for k in range(4):
            dma(w1T_sb[k][:], w1T[ts(k, 128), :])
            dma(w2T_sb[k][:], w2T[ts(k, 128), :])
        g_sb1 = load_const(fconst, g_row, [1, E2])
        bl_sb1 = load_const(fconst, bl_row, [1, E2])
        gb_ps = ps512([128, E2], "gb_ps")
        for cc in range(E2 // 512):
            nc.tensor.matmul(gb_ps[:, ts(cc, 512)], ones_sb[:, :],
                             g_sb1[:, ts(cc, 512)], start=True, stop=True)
        g_bc = fconst.tile([128, E2], FP, tag="g_bc")
        nc.vector.tensor_copy(g_bc[:], gb_ps[:])
        bb_ps = ps512([128, E2], "bb_ps")
        for cc in range(E2 // 512):
            nc.tensor.matmul(bb_ps[:, ts(cc, 512)], ones_sb[:, :],
                             bl_sb1[:, ts(cc, 512)], start=True, stop=True)
        b_bc = fconst.tile([128, E2], FP, tag="b_bc")
        nc.vector.tensor_copy(b_bc[:], bb_ps[:])

        # side 0 uses m from orientation o=1 (m0); side 1 uses o=0 (m1)
        moT = {}
        for s in (0, 1):
            o = 1 - s
            for p in range(2):
                mo_sb = mo_pool.tile([128, NHF], FP, tag=f"mo{s}{p}",
                                     name=f"mo{s}{p}", bufs=1)
                for cc in range(NMC):
                    ps = ps512([128, MCH], "mops")
                    for k in range(2):
                        nc.tensor.matmul(
                            ps[:], woT_sb[k][:, ts(p, 128)],
                            mt[(o, k)][:, ts(cc, MCH)],
                            start=(k == 0), stop=(k == 1))
                    nc.scalar.activation(
                        mo_sb[:, ts(cc, MCH)], ps[:], AF.Identity,
                        bias=bo_sb[p][:], scale=1.0)
                moT[(s, p)] = mo_sb

        # sweep A: H = W1 @ [x;m] + b1 (token-major), center, rstd
        h_sb = {}
        rstd = {}
        for s in (0, 1):
            ctk = [xto[(s, 0)], xto[(s, 1)], moT[(s, 0)], moT[(s, 1)]]
            for j in range(RT):
                hp = ps512([128, E2], "hp")
                nc.tensor.matmul(hp[:, 0:E2], ones_sb[:, :], b1_sb[:, 0:E2],
                                 start=True, stop=False)
                for k in range(4):
                    nc.tensor.matmul(hp[:, 0:E2], ctk[k][:, ts(j, 128)],
                                     w1T_sb[k][:], start=False, stop=(k == 3))
                st6 = stat.tile([128, 6], FP, tag="st6", name="st6")
                nc.vector.bn_stats(st6[:], hp[:])
                mv = stat.tile([128, 2], FP, tag="mv", name="mv")
                nc.vector.bn_aggr(mv[:], st6[:])
                hs = h_pool.tile([128, E2], FP, tag=f"h{s}{j}",
                                 name=f"h{s}{j}", bufs=1)
                nc.vector.tensor_scalar(
                    hs[:], hp[:], mv[:, 0:1], None, op0=ALU.subtract)
                h_sb[(s, j)] = hs
                sq = stat.tile([128, 1], FP, tag="sq", name="sq")
                nc.scalar.activation(sq[:], mv[:, 1:2], AF.Sqrt, bias=eps_sb[:])
                rc = stat.tile([128, 1], FP, tag=f"rstd{s}{j}",
                               name=f"rstd{s}{j}", bufs=1)
                nc.vector.reciprocal(rc[:], sq[:])
                rstd[(s, j)] = rc

        # sweep B: scale*g+b, gelu, transpose, W2, +x, store
        for s in (0, 1):
            for j in range(RT):
                hn = hg_pool.tile([128, E2], FP, tag="hn", name="hn")
                nc.vector.tensor_scalar_mul(hn[:], h_sb[(s, j)][:],
                                            rstd[(s, j)][:])
                nc.vector.tensor_mul(hn[:], hn[:], g_bc[:])
                nc.vector.tensor_add(hn[:], hn[:], b_bc[:])
                hg = hg_pool.tile([128, E2], FP, tag="hg", name="hg")
                nc.scalar.activation(hg[:], hn[:], gelu_func)
                hgt = [hgt_pool.tile([128, 128], FP, tag=f"hgt{k}",
                                     name=f"hgt{k}") for k in range(4)]
                for k in range(4):
                    tp = ps512([128, 128], "tp2")
                    nc.tensor.transpose(tp[:], hg[:, ts(k, 128)], id_sb[:])
                    nc.scalar.copy(hgt[k][:], tp[:])
                op = ps512([128, E], "op")
                nc.tensor.matmul(op[:], ones_sb[:, :], b2_sb[:, :],
                                 start=True, stop=False)
                for k in range(4):
                    nc.tensor.matmul(op[:], hgt[k][:], w2T_sb[k][:],
                                     start=False, stop=(k == 3))
                xres = xload.tile([128, E], FP, tag="xl", name="xres")
                dma(xres[:], xin[(s, "own")][ts(j, 128), :])
                yo = o_pool.tile([128, E], FP, tag="yo", name="yo")
                nc.vector.tensor_add(yo[:], op[:], xres[:])
                dma(y_out[s][ts(j, 128), :], yo[:])
        ffn_st.close()

    if split_waits:
        _split_matmul_waits(nc)
    return nc


_SPLIT_TYPES = ("InstMatmult", "InstDMACopy", "InstDMATranspose")


def _split_matmul_waits(nc, max_waits=1):
    """walrus (this build) rejects Matmult instructions carrying more than one
    sync wait (S3_LW struct). Strip waits off such matmuls onto single-wait
    EventSemaphore instructions inserted just before them on the same engine."""
    esn = [0]
    for blk in nc.m.functions[0].blocks:
        insts = blk.instructions   # live list
        i = 0
        while i < len(insts):
            inst = insts[i]
            si = inst.sync_info
            if si is not None and len(si.on_wait) > max_waits:
                waits = list(si.on_wait)
                inst.sync_info = mybir.SyncInfo(on_wait=waits[-1:],
                                                on_update=list(si.on_update))
                for w in waits[:-1]:
                    es = mybir.InstEventSemaphore(name=f"I-esw{esn[0]}", ins=[], outs=[])
                    esn[0] += 1
                    es.engine = inst.engine
                    es.sync_info = mybir.SyncInfo(on_wait=[w], on_update=[])
                    insts.insert(i, es)
                    i += 1
            i += 1
    return nc


_CACHE = {}


def _get_nc(NT):
    if NT not in _CACHE:
        _CACHE[NT] = build_program(NT)
    return _CACHE[NT]


def make_in_maps(inputs, NT=N_FULL, n_cores=8):
    """Host-side sharding: per-core input dicts."""
    f32 = lambda a: np.ascontiguousarray(np.asarray(a, dtype=np.float32))
    x0, x1 = f32(inputs["x0"]), f32(inputs["x1"])
    Wqk, bqk = f32(inputs["Wqk"]), f32(inputs["bqk"])
    Wv, bv = f32(inputs["Wv"]), f32(inputs["bv"])
    Wo, bo = f32(inputs["Wo"]), f32(inputs["bo"])
    W1, b1 = f32(inputs["W1"]), f32(inputs["b1"])
    g_ln, b_ln = f32(inputs["g_ln"]), f32(inputs["b_ln"])
    W2, b2 = f32(inputs["W2"]), f32(inputs["b2"])
    NHF = NT // 2
    shared = dict(
        wqkT=f32(Wqk.T), wvT=f32(Wv.T), woT=f32(Wo.T),
        w1T=f32(W1.T), w2T=f32(W2.T),
        bqkm=f32((bqk * MULT).reshape(E, 1)),
        bv_row=f32(bv.reshape(1, E)),
        bo_col=f32(bo.reshape(E, 1)),
        b1_row=f32(b1.reshape(1, E2)),
        b2_row=f32(b2.reshape(1, E)),
        g_row=f32(g_ln.reshape(1, E2)),
        bl_row=f32(b_ln.reshape(1, E2)),
        ones_row=np.ones((1, 128), np.float32),
        eps_col=np.full((128, 1), EPS, np.float32),
        ident=np.eye(128, dtype=np.float32),
    )
    in_maps = []
    for c in range(n_cores):
        b, hf = divmod(c, 2)
        own = slice(hf * NHF, (hf + 1) * NHF)
        oth = slice((1 - hf) * NHF, (2 - hf) * NHF)
        m = dict(shared)
        m["x0_own"] = f32(x0[b, own])
        m["x0_oth"] = f32(x0[b, oth])
        m["x1_own"] = f32(x1[b, own])
        m["x1_oth"] = f32(x1[b, oth])
        in_maps.append(m)
    return in_maps


def assemble(results, NT=N_FULL, n_cores=8):
    """Reassemble per-core outputs into full arrays."""
    NHF = NT // 2
    out0 = np.empty((B, NT, E), np.float32)
    out1 = np.empty((B, NT, E), np.float32)
    a01 = np.empty((B, H, NT, NT), np.float32)
    a10 = np.empty((B, H, NT, NT), np.float32)
    for c in range(n_cores):
        b, hf = divmod(c, 2)
        own = slice(hf * NHF, (hf + 1) * NHF)
        oth = slice((1 - hf) * NHF, (2 - hf) * NHF)
        r = results[c]
        out0[b, own] = r["out0_o"]
        out1[b, own] = r["out1_o"]
        a01[b, :, own, own] = r["attn01_o"][:, :, :NHF]
        a01[b, :, own, oth] = r["attn01_o"][:, :, NHF:]
        a10[b, :, own, own] = r["attn10_o"][:, :, :NHF]
        a10[b, :, own, oth] = r["attn10_o"][:, :, NHF:]
    return out0, out1, a01, a10


LAST_RUN = None


def kernel(**inputs):
    global LAST_RUN
    os.environ.setdefault("MYCRO_LOCAL_CACHE", "1")
    from concourse.bass_utils import run_bass_kernel_spmd
    nc = _get_nc(N_FULL)
    in_maps = make_in_maps(inputs, N_FULL, 8)
    res = run_bass_kernel_spmd(nc, in_maps, core_ids=list(range(8)))
    LAST_RUN = res
    return assemble(res.results, N_FULL, 8)
